# revision 1
# baseline (speedup 1.0000x reference)
"""Trainium2 Bass kernel for nn_BitwiseMultipyLogis (gnn_message_passing).

Reference computation (L=8 layers, N=100000 nodes, F=128 features):
    proj    = tanh(node_features @ trans + bias)          # [L, N, F]
    bitwise = proj * proj[layer_predict]                  # [L, N, F]
    bitwise = einsum('lnf,lfg->lng', bitwise, theta)      # [L, N, F]
    scores  = sigmoid(bitwise @ logis_w[0] + logis_b)     # [L, N]
    weights = softmax(scores, axis=0)                     # [L, N]
    out     = proj[layer_predict] + sum_l weights[l]*proj[l]   # [N, F]

Key algebraic simplification: theta only feeds the logis_w dot product, so
    scores[l,n] = sigmoid( sum_f proj[l,n,f]*proj[lp,n,f]*v[l,f] + logis_b )
with v[l] = theta[l] @ logis_w[0] precomputed on host.  This removes the
entire [L,N,F]x[L,F,F] einsum (half the FLOPs).

Device strategy (8 NeuronCores, data-parallel over N, 12500 nodes/core):
  * transposed layout [F=128 partitions, node columns]; host pre-transposes
    node_features to [L, ntiles, 128, 512] bf16 so every DMA is contiguous.
  * projT = trans^T @ xT on TensorE (bf16, f32 PSUM), tanh+bias on ScalarE.
  * scores via accumulated matmuls with one-hot-masked v columns: 8 matmuls
    lhsT=v8oh[l] produce the full [8, n] score tile in one PSUM bank.
  * sigmoid+softmax without table swap: sigmoid(x)=(1+tanh(x/2))/2 and
    exp(sigmoid(x)) = exp(0.5*tanh(x/2) + 0.5)  (Tanh and Exp share the
    ScalarE table set; softmax max-subtraction is safe to skip since
    sigmoid outputs are in (0,1)).
  * softmax denominator via ones8 matmul; weights broadcast across the 128
    partitions with GpSimd partition_broadcast; weighted sum accumulated in
    PSUM via identity matmuls.
  * host transposes the [128, n] result back to [n, 128].
"""

import os
import numpy as np
import ml_dtypes
from contextlib import ExitStack

import concourse.bass as bass
import concourse.mybir as mybir
import concourse.tile as tile
from concourse import bacc
from concourse.bass import ts

BF16 = mybir.dt.bfloat16
F32 = mybir.dt.float32
AF = mybir.ActivationFunctionType

L, N, F = 8, 100000, 128
CORES = 8
NS = N // CORES            # 12500 nodes per core
TILE = 512                 # node columns per tile (one f32 PSUM bank)
NT = (NS + TILE - 1) // TILE   # 25
NSP = NT * TILE            # 12800 (padded)

BF = ml_dtypes.bfloat16


def _body(tc, out, ins, lp: int, logis_b: float, nt: int, dbg=None):
    """Emit the tile program.  out: [128, nt*TILE] dram AP; ins: dict of APs.
    dbg: optional dict of dram APs receiving tile-0 intermediates."""
    nc = tc.nc
    with ExitStack() as ctx:
        const = ctx.enter_context(tc.tile_pool(name="const", bufs=1))
        xts = ctx.enter_context(tc.tile_pool(name="xts", bufs=2))
        projp = ctx.enter_context(tc.tile_pool(name="projp", bufs=2, space="PSUM"))
        projs = ctx.enter_context(tc.tile_pool(name="projs", bufs=2))
        bits = ctx.enter_context(tc.tile_pool(name="bits", bufs=2))
        scp = ctx.enter_context(tc.tile_pool(name="scp", bufs=1, space="PSUM"))
        scs = ctx.enter_context(tc.tile_pool(name="scs", bufs=2))
        wbp = ctx.enter_context(tc.tile_pool(name="wbp", bufs=2, space="PSUM"))  # shared agg/wb/rb tag
        ys = ctx.enter_context(tc.tile_pool(name="ys", bufs=2))
        sump = ctx.enter_context(tc.tile_pool(name="sump", bufs=1, space="PSUM"))
        outs = ctx.enter_context(tc.tile_pool(name="outs", bufs=2))

        trans_sb = const.tile([128, 128], BF16)
        nc.sync.dma_start(trans_sb[:], ins["trans"])
        # v8sp: per layer l a [128, 128] one-hot-column matrix whose column
        # 32*(l%3) holds v[l]; used as lhsT so layer l's score row lands at
        # partition 32*(l%3) of score group l//3 (base partitions are limited
        # to {0,32,64} for later rhs reads, so 3 layers per PSUM bank).
        v8sp_sb = const.tile([128, L * 128], BF16)
        nc.sync.dma_start(v8sp_sb[:], ins["v8sp"])
        ident_sb = const.tile([128, 128], BF16)
        nc.sync.dma_start(ident_sb[:], ins["ident"])
        # selection columns: col0 = ones at {0,32,64}, col1 = ones at {0,32}
        sel32_sb = const.tile([128, 2], BF16)
        nc.sync.dma_start(sel32_sb[:], ins["sel32"])
        # all-ones rows: K=1 lhsT that replicates a [1, n] rhs row across
        # all 128 output partitions (PE-based partition broadcast).  Full
        # [128, 128] so a row at the rhs's base partition can be sliced
        # (bass requires lhsT.base_partition == rhs.base_partition).
        onesr_sb = const.tile([128, 128], BF16)
        nc.sync.dma_start(onesr_sb[:], ins["onesr"])
        onesr32_sb = const.tile([128, 128], F32)
        nc.sync.dma_start(onesr32_sb[:], ins["onesr32"])
        bias_sb = const.tile([128, 1], F32)
        nc.sync.dma_start(bias_sb[:], ins["biasc"])
        # small constant biases for the score activations (activation() only
        # auto-converts float biases that are pre-registered const APs)
        lb_bias = const.tile([128, 1], F32)
        nc.gpsimd.memset(lb_bias[:], 0.5 * logis_b)
        half_bias = const.tile([128, 1], F32)
        nc.gpsimd.memset(half_bias[:], 0.5)

        xt = ins["xt"]
        for t in range(nt):
            xt_sb = xts.tile([128, L, TILE], BF16, tag="xt")
            for l in range(L):
                nc.sync.dma_start(xt_sb[:, l, :], xt[l, t])

            # projT[l] = tanh(trans^T @ xT[l] + bias)   [128f, TILE]
            proj = projs.tile([128, L, TILE], BF16, tag="proj")
            for l in range(L):
                pp = projp.tile([128, TILE], F32, tag="pp")
                nc.tensor.matmul(pp[:], trans_sb[:], xt_sb[:, l, :],
                                 start=True, stop=True)
                nc.scalar.activation(proj[:, l, :], pp[:], AF.Tanh,
                                     bias=bias_sb[:, 0:1], scale=1.0)

            # bit[l] = projT[l] * projT[lp]
            bit = bits.tile([128, L, TILE], BF16, tag="bit")
            for l in range(L):
                nc.vector.tensor_mul(bit[:, l, :], proj[:, l, :], proj[:, lp, :])

            # scores_raw[l, n] = sum_f v[l,f] * bit[l,f,n].  Layer l's score
            # row lands at partition 32*(l%3) of score group l//3: groups 0/1
            # in the two banks of sc_psA, group 2 (layers 6,7) in sc_psB.
            # All MMs of a group cover the same region so the start=True MM
            # clears every has_written bit the group touches.
            expvs = []
            for g in range(3):
                nls = 3 if g < 2 else 2
                m = 32 * (nls - 1) + 1
                sc_ps = scp.tile([128, TILE], F32, tag=f"scps{g}")
                for s in range(nls):
                    l = 3 * g + s
                    nc.tensor.matmul(
                        sc_ps[0:m, :],
                        v8sp_sb[:, l * 128 : l * 128 + m],
                        bit[:, l, :],
                        start=(s == 0), stop=(s == nls - 1),
                    )
                # e = exp(sigmoid(raw + lb)) with no table swap:
                # t = tanh(0.5*raw + 0.5*lb); e = exp(0.5*t + 0.5)
                sct = scs.tile([128, TILE], F32, tag=f"sct{g}")
                nc.scalar.activation(sct[0:m, :], sc_ps[0:m, :], AF.Tanh,
                                     bias=lb_bias[0:m, :], scale=0.5)
                expv = scs.tile([128, TILE], BF16, tag=f"expv{g}")
                nc.scalar.activation(expv[0:m, :], sct[0:m, :], AF.Exp,
                                     bias=half_bias[0:m, :], scale=0.5)
                expvs.append(expv)

            def _erow(l):
                g, s = divmod(l, 3)
                return expvs[g][32 * s : 32 * s + 1, :]

            # sumexp + reciprocal
            se_ps = sump.tile([1, TILE], F32, tag="seps")
            nc.tensor.matmul(se_ps[:], sel32_sb[0:65, 0:1], expvs[0][0:65, :],
                             start=True, stop=False)
            nc.tensor.matmul(se_ps[:], sel32_sb[0:65, 0:1], expvs[1][0:65, :],
                             start=False, stop=False)
            nc.tensor.matmul(se_ps[:], sel32_sb[0:33, 1:2], expvs[2][0:33, :],
                             start=False, stop=True)
            rec = scs.tile([1, TILE], F32, tag="rec")
            nc.vector.reciprocal(rec[:], se_ps[:])

            # y[l] = projT[l] * e_bcast[l];  agg = sum_l y[l]  (identity MMs).
            # Each weight row is replicated across partitions by a K=1 matmul
            # with an all-ones lhsT row (PE-based broadcast into PSUM).
            y = ys.tile([128, L, TILE], BF16, tag="y")
            for l in range(L):
                wb = wbp.tile([128, TILE], F32, tag="wagg")
                q = 32 * (l % 3)
                nc.tensor.matmul(wb[:], onesr_sb[q : q + 1, :], _erow(l),
                                 start=True, stop=True)
                nc.vector.tensor_mul(y[:, l, :], proj[:, l, :], wb[:])
            agg = wbp.tile([128, TILE], F32, tag="wagg")
            for l in range(L):
                nc.tensor.matmul(agg[:], ident_sb[:], y[:, l, :],
                                 start=(l == 0), stop=(l == L - 1))

            # out = projT[lp] + agg * recip_bcast
            rb = wbp.tile([128, TILE], F32, tag="wagg")
            nc.tensor.matmul(rb[:], onesr32_sb[0:1, :], rec[:],
                             start=True, stop=True)
            rb_sb = outs.tile([128, TILE], BF16, tag="rbsb")
            nc.vector.tensor_copy(rb_sb[:], rb[:])
            nrm = outs.tile([128, TILE], BF16, tag="nrm")
            nc.vector.tensor_mul(nrm[:], agg[:], rb_sb[:])
            ot = outs.tile([128, TILE], BF16, tag="ot")
            nc.vector.tensor_add(ot[:], nrm[:], proj[:, lp, :])
            nc.sync.dma_start(out[:, ts(t, TILE)], ot[:])

            if dbg is not None and t == 0:
                nc.sync.dma_start(dbg["a_proj"][:], proj[:])
                nc.sync.dma_start(dbg["b_bit"][:], bit[:])
                nc.sync.dma_start(dbg["c_expv0"][:], expvs[0][0:65, :])
                nc.sync.dma_start(dbg["d_rec"][:], rec[:])
                nc.sync.dma_start(dbg["g_y"][:], y[:])
                nc.sync.dma_start(dbg["h_nrm"][:], nrm[:])


def _build(lp: int, logis_b: float, nt: int = NT, reps: int = 1):
    nc = bacc.Bacc("TRN2", target_bir_lowering=False, debug=False,
                   num_devices=CORES)
    ins = {
        "xt": nc.dram_tensor("xt", [L, nt, 128, TILE], BF16,
                             kind="ExternalInput").ap(),
        "trans": nc.dram_tensor("trans", [128, 128], BF16,
                                kind="ExternalInput").ap(),
        "v8sp": nc.dram_tensor("v8sp", [128, L * 128], BF16,
                               kind="ExternalInput").ap(),
        "ident": nc.dram_tensor("ident", [128, 128], BF16,
                                kind="ExternalInput").ap(),
        "sel32": nc.dram_tensor("sel32", [128, 2], BF16,
                                kind="ExternalInput").ap(),
        "onesr": nc.dram_tensor("onesr", [128, 128], BF16,
                                kind="ExternalInput").ap(),
        "onesr32": nc.dram_tensor("onesr32", [128, 128], F32,
                                  kind="ExternalInput").ap(),
        "biasc": nc.dram_tensor("biasc", [128, 1], F32,
                                kind="ExternalInput").ap(),
    }
    out = nc.dram_tensor("out", [128, nt * TILE], BF16,
                         kind="ExternalOutput").ap()
    with tile.TileContext(nc) as tc:
        if reps == 1:
            _body(tc, out, ins, lp, logis_b, nt)
        else:
            with tc.For_i(0, reps, 1):
                _body(tc, out, ins, lp, logis_b, nt)
    nc.compile()
    return nc


def _host_prep(inputs):
    nf = np.asarray(inputs["node_features"], np.float32)      # [L, N, F]
    trans = np.asarray(inputs["trans"], np.float32)           # [F, F]
    biasv = np.asarray(inputs["bias"], np.float32).reshape(F) # [F]
    theta = np.asarray(inputs["theta"], np.float32)           # [L, F, F]
    lw = np.asarray(inputs["logis_w"], np.float32).reshape(1, F)
    lb = float(np.asarray(inputs["logis_b"], np.float32).reshape(-1)[0])
    lp = int(np.asarray(inputs["layer_predict"]).reshape(-1)[0])

    v8 = theta @ lw[0]                                        # [L, F]
    v8sp = np.zeros((128, L * 128), np.float32)
    for l in range(L):
        v8sp[:, l * 128 + 32 * (l % 3)] = v8[l]

    sel32 = np.zeros((128, 2), np.float32)
    sel32[[0, 32, 64], 0] = 1.0
    sel32[[0, 32], 1] = 1.0

    consts = {
        "trans": trans.astype(BF),
        "v8sp": v8sp.astype(BF),
        "ident": np.eye(128, dtype=np.float32).astype(BF),
        "sel32": sel32.astype(BF),
        "onesr": np.ones((128, 128), np.float32).astype(BF),
        "onesr32": np.ones((128, 128), np.float32),
        "biasc": np.ascontiguousarray(biasv.reshape(128, 1)),
    }

    # per-core transposed, padded, tiled node features
    nfb = nf.astype(BF)                                       # [L, N, F]
    in_maps = []
    for c in range(CORES):
        sl = nfb[:, c * NS : (c + 1) * NS, :]                 # [L, NS, F]
        xt = np.transpose(sl, (0, 2, 1))                      # [L, F, NS]
        if NSP != NS:
            xt = np.concatenate(
                [xt, np.zeros((L, F, NSP - NS), BF)], axis=2)
        xt = np.ascontiguousarray(
            np.transpose(xt.reshape(L, F, NT, TILE), (0, 2, 1, 3)))
        in_maps.append({"xt": xt, **consts})
    return in_maps, lp, lb


_cache = {}


def _run(inputs, trace=False):
    from concourse.bass_utils import run_bass_kernel_spmd

    in_maps, lp, lb = _host_prep(inputs)
    key = lp
    if key not in _cache:
        _cache[key] = _build(lp, lb)
    nc = _cache[key]

    res = run_bass_kernel_spmd(nc, in_maps, core_ids=list(range(CORES)),
                               trace=trace)
    parts = []
    for c in range(CORES):
        o = np.asarray(res.results[c]["out"], dtype=np.float32)  # [128, NSP]
        parts.append(o[:, :NS].T)                                # [NS, 128]
    full = np.concatenate(parts, axis=0)                         # [N, F]
    return full, res


def kernel(**inputs) -> np.ndarray:
    out, _ = _run(inputs, trace=False)
    return out


def timed_run(inputs, reps=17, nruns=5):
    """On-device timing: build a variant whose whole tile program runs
    `reps` times inside a tc.For_i loop, keep inputs device-resident, and
    difference against the 1-rep program to cancel dispatch round-trip.

    Returns (per_exec_ns, times_dict).
    """
    import time
    import jax
    import jax.numpy as jnp
    from jax.sharding import Mesh, PartitionSpec, NamedSharding
    from jax.experimental.shard_map import shard_map
    import concourse.bass2jax as b2j
    from concourse import mybir as _mb

    in_maps, lp, lb = _host_prep(inputs)
    b2j.install_neuronx_cc_hook()

    devices = jax.devices()[:CORES]
    mesh = Mesh(np.asarray(devices), ("core",))
    sh = NamedSharding(mesh, PartitionSpec("core"))

    def make_jit(nc):
        in_names, out_names, out_avals, zero_outs = [], [], [], []
        for alloc in nc.m.functions[0].allocations:
            if not isinstance(alloc, _mb.MemoryLocationSet):
                continue
            name = alloc.memorylocations[0].name
            if alloc.kind == "ExternalInput":
                in_names.append(name)
            elif alloc.kind == "ExternalOutput":
                out_names.append(name)
                shape = tuple(alloc.tensor_shape)
                dtype = _mb.dt.np(alloc.dtype)
                out_avals.append(jax.core.ShapedArray(shape, dtype))
                zero_outs.append(np.zeros(shape, dtype))
        n_params = len(in_names)
        all_names = in_names + out_names

        def _bodyf(*args):
            outs = b2j._bass_exec_p.bind(
                *args,
                out_avals=tuple(out_avals),
                in_names=tuple(all_names),
                out_names=tuple(out_names),
                lowering_input_output_aliases=(),
                sim_require_finite=True,
                sim_require_nnan=True,
                nc=nc,
            )
            return tuple(outs)

        n_outs = len(out_names)
        donate = tuple(range(n_params, n_params + n_outs))
        f = jax.jit(
            shard_map(_bodyf, mesh=mesh,
                      in_specs=(PartitionSpec("core"),) * (n_params + n_outs),
                      out_specs=(PartitionSpec("core"),) * n_outs,
                      check_rep=False),
            donate_argnums=donate, keep_unused=True,
        )
        return f, in_names, zero_outs

    key = (lp, round(lb, 8))
    if key not in _cache:
        _cache[key] = _build(lp, lb)
    nc1 = _cache[key]
    ncR = _build(lp, lb, reps=reps)

    results = {}
    for tag, nc in (("r1", nc1), ("rR", ncR)):
        f, in_names, zero_outs = make_jit(nc)
        concat_in = [
            np.concatenate([np.asarray(in_maps[c][nm]) for c in range(CORES)],
                           axis=0)
            for nm in in_names
        ]
        dev_in = [jax.device_put(a, sh) for a in concat_in]

        def one_run():
            zs = [jax.device_put(
                jnp.zeros((CORES * z.shape[0], *z.shape[1:]), z.dtype), sh)
                for z in zero_outs]
            jax.block_until_ready(zs)
            t0 = time.perf_counter()
            o = f(*dev_in, *zs)
            jax.block_until_ready(o)
            return time.perf_counter() - t0

        one_run()  # compile + warmup
        best = min(one_run() for _ in range(nruns))
        results[tag] = best

    per_exec = (results["rR"] - results["r1"]) / (reps - 1)
    return per_exec * 1e9, results



# revision 6
# speedup vs baseline: 14574.8567x; 14574.8567x over previous
"""Trainium2 Bass kernel for nn_BitwiseMultipyLogis (gnn_message_passing).

Reference computation (L=8 layers, N=100000 nodes, F=128 features):
    proj    = tanh(node_features @ trans + bias)          # [L, N, F]
    bitwise = proj * proj[layer_predict]                  # [L, N, F]
    bitwise = einsum('lnf,lfg->lng', bitwise, theta)      # [L, N, F]
    scores  = sigmoid(bitwise @ logis_w[0] + logis_b)     # [L, N]
    weights = softmax(scores, axis=0)                     # [L, N]
    out     = proj[layer_predict] + sum_l weights[l]*proj[l]   # [N, F]

Algebraic simplification: theta only feeds the logis_w dot product, so
    scores[l,n] = sigmoid( sum_f proj[l,n,f]*proj[lp,n,f]*v[l,f] + logis_b )
with v[l] = theta[l] @ logis_w[0] precomputed on host.  This removes the
entire [L,N,F]x[L,F,F] einsum (half the FLOPs).

Device strategy (8 NeuronCores, data-parallel over N, 12500 nodes/core),
v3 — engine-balanced, instruction-count-minimized:
  * transposed layout [F=128 partitions, node columns]; host pre-packs
    node_features to [NT, 128, L, 512] bf16 so ONE DMA loads a whole tile.
  * projT = trans^T @ xT on TensorE (bf16), tanh+bias on ScalarE.
  * all 8 score rows accumulate into ONE [8, 512] PSUM tile via one-hot
    v columns; sigmoid+softmax with no table swap:
    exp(sigmoid(x)) = exp(0.5*tanh(x/2) + 0.5)  -> 2 activations per tile.
  * softmax denominators of a 4-tile group accumulate into one [97, 512]
    PSUM tile (rows 0/32/64/96) so ONE DVE reciprocal serves 4 tiles.
  * weights normalized as rows: w8 = e8 * bcast8(recip); the projlp term
    is folded into layer lp's y-multiply via scalar_tensor_tensor
    (y_lp = (wb_lp + 1) * proj_lp), so no final add is needed.
  * weight broadcast rows->128 partitions via [8,128] row-ones lhsT.
  * aggregation: 32 transpose-accumulate matmuls sum y tiles directly into
    a NODE-MAJOR bf16 PSUM tile [128, 4, 128]; ScalarE copies it to SBUF
    and 4 DMAs store [128 nodes, 128 f] blocks.  Output is node-major so
    the host does no transpose.
"""

import numpy as np
import ml_dtypes
from contextlib import ExitStack
from concurrent.futures import ThreadPoolExecutor

import concourse.bass as bass
import concourse.mybir as mybir
import concourse.tile as tile
from concourse import bacc
from concourse.bass import ts

BF16 = mybir.dt.bfloat16
F32 = mybir.dt.float32
AF = mybir.ActivationFunctionType
ALU = mybir.AluOpType

L, N, F = 8, 100000, 128
CORES = 8
NS = N // CORES            # 12500 nodes per core
TILE = 512                 # node columns per tile (one f32 PSUM bank)
NT = (NS + TILE - 1) // TILE   # 25
NSP = NT * TILE            # 12800 (padded)
GRP = 3                    # tiles per reciprocal group (rows 0/32/64)
NB = TILE // 128           # 128-node blocks per tile

BF = ml_dtypes.bfloat16


def _body(tc, out, ins, lp: int, logis_b: float, nt: int):
    nc = tc.nc
    with ExitStack() as ctx:
        const = ctx.enter_context(tc.tile_pool(name="const", bufs=1))
        xts = ctx.enter_context(tc.tile_pool(name="xts", bufs=2 * GRP))
        projp = ctx.enter_context(tc.tile_pool(name="projp", bufs=2, space="PSUM"))
        projs = ctx.enter_context(tc.tile_pool(name="projs", bufs=GRP + 2))
        bits = ctx.enter_context(tc.tile_pool(name="bits", bufs=3))
        scp = ctx.enter_context(tc.tile_pool(name="scp", bufs=1, space="PSUM"))
        scs = ctx.enter_context(tc.tile_pool(name="scs", bufs=2))
        e8s = ctx.enter_context(tc.tile_pool(name="e8s", bufs=GRP + 2))
        sep = ctx.enter_context(tc.tile_pool(name="sep", bufs=1, space="PSUM"))
        rcs = ctx.enter_context(tc.tile_pool(name="rcs", bufs=2))
        rc8p = ctx.enter_context(tc.tile_pool(name="rc8p", bufs=1, space="PSUM"))
        w8s = ctx.enter_context(tc.tile_pool(name="w8s", bufs=2))
        wbp = ctx.enter_context(tc.tile_pool(name="wbp", bufs=2, space="PSUM"))
        ys = ctx.enter_context(tc.tile_pool(name="ys", bufs=3))
        aggp = ctx.enter_context(tc.tile_pool(name="aggp", bufs=1, space="PSUM"))
        outs = ctx.enter_context(tc.tile_pool(name="outs", bufs=2))

        trans_sb = const.tile([128, 128], BF16)
        nc.sync.dma_start(trans_sb[:], ins["trans"])
        # v8oh[:, l, :]: [128, 8] one-hot lhsT; column l holds v[l] so the
        # score row of layer l lands at PSUM partition l.
        v8oh_sb = const.tile([128, L, 8], BF16)
        nc.sync.dma_start(v8oh_sb[:], ins["v8oh"])
        # rowones[:, l, :]: [8, 128] lhsT with row l all-ones: broadcasts
        # weight row l across all 128 output partitions.
        rowones_sb = const.tile([8, L, 128], BF16)
        nc.sync.dma_start(rowones_sb[:], ins["rowones"])
        # sumsel[:, k, :]: [8, 97] lhsT, ones in column 32k only: the layer
        # sum of tile k-of-group lands at PSUM partition 32k.
        sumsel_sb = const.tile([8, GRP, 65], BF16)
        nc.sync.dma_start(sumsel_sb[:], ins["sumsel"])
        onesrow_sb = const.tile([128, 8], BF16)
        nc.sync.dma_start(onesrow_sb[:], ins["onesrow"])
        ident_sb = const.tile([128, 128], BF16)
        nc.sync.dma_start(ident_sb[:], ins["ident"])
        bias_sb = const.tile([128, 1], F32)
        nc.sync.dma_start(bias_sb[:], ins["biasc"])
        lb_bias = const.tile([128, 1], F32)
        nc.gpsimd.memset(lb_bias[:], 0.5 * logis_b)
        half_bias = const.tile([128, 1], F32)
        nc.gpsimd.memset(half_bias[:], 0.5)
        zbias = const.tile([128, 1], F32)
        nc.gpsimd.memset(zbias[:], 0.0)

        xt = ins["xt"]
        ngroups = (nt + GRP - 1) // GRP
        xt_tiles = {}

        def issue_in_dma(t):
            if t < nt:
                xt_sb = xts.tile([128, L, TILE], BF16, tag="xt")
                nc.sync.dma_start(xt_sb[:], xt[t])
                xt_tiles[t] = xt_sb

        for t in range(min(GRP, nt)):
            issue_in_dma(t)

        for g in range(ngroups):
            t0 = g * GRP
            gn = min(GRP, nt - t0)
            projs_g, e8_g = {}, {}
            seP = sep.tile([65, TILE], F32, tag="seP")

            # ---- phase A: proj, bits, scores, exp, denominator ----
            for k in range(gn):
                t = t0 + k
                xt_sb = xt_tiles.pop(t)
                proj = projs.tile([128, L, TILE], BF16, tag="proj")
                projs_g[t] = proj
                for l in range(L):
                    pp = projp.tile([128, TILE], F32, tag="pp")
                    nc.tensor.matmul(pp[:], trans_sb[:], xt_sb[:, l, :],
                                     start=True, stop=True)
                    nc.scalar.activation(proj[:, l, :], pp[:], AF.Tanh,
                                         bias=bias_sb[:, 0:1], scale=1.0)
                sc = scp.tile([8, TILE], F32, tag="sc")
                for l in range(L):
                    bit = bits.tile([128, TILE], BF16, tag="bit")
                    eng = nc.gpsimd if l in (3, 7) else nc.vector
                    eng.tensor_mul(bit[:], proj[:, l, :], proj[:, lp, :])
                    nc.tensor.matmul(sc[:], v8oh_sb[:, l, :], bit[:],
                                     start=(l == 0), stop=(l == L - 1))
                # e = exp(sigmoid(raw + lb)) with no table swap:
                # u = tanh(0.5*raw + 0.5*lb); e = exp(0.5*u + 0.5)
                sct = scs.tile([8, TILE], F32, tag="sct")
                nc.scalar.activation(sct[:], sc[:], AF.Tanh,
                                     bias=lb_bias[0:8, :], scale=0.5)
                e8 = e8s.tile([8, TILE], BF16, tag="e8")
                nc.scalar.activation(e8[:], sct[:], AF.Exp,
                                     bias=half_bias[0:8, :], scale=0.5)
                e8_g[t] = e8
                nc.tensor.matmul(seP[:], sumsel_sb[:, k, :], e8[:],
                                 start=(k == 0), stop=(k == gn - 1))

            # prefetch next group's inputs before this group's out DMAs
            for k in range(GRP):
                issue_in_dma(t0 + GRP + k)

            # ---- phase B: batched reciprocal of the 4 denominators ----
            recf = rcs.tile([65, TILE], F32, tag="recf")
            nc.vector.reciprocal(recf[:], seP[:])
            recb = rcs.tile([65, TILE], BF16, tag="recb")
            nc.vector.tensor_copy(recb[:], recf[:])

            # ---- phase C: normalize weights, broadcast, weighted agg ----
            for k in range(gn):
                t = t0 + k
                proj = projs_g[t]
                q = 32 * k
                rec8 = rc8p.tile([8, TILE], F32, tag="rec8")
                nc.tensor.matmul(rec8[:], onesrow_sb[q:q + 1, :],
                                 recb[q:q + 1, :], start=True, stop=True)
                w8 = w8s.tile([8, TILE], BF16, tag="w8")
                nc.vector.tensor_mul(w8[:], e8_g[t][:], rec8[:])
                # f32 PSUM accumulator; the lp fold makes this the
                # final result: agg = sum_l y[l] = out (f-major)
                agg = aggp.tile([128, TILE], F32, tag="agg")
                for l in range(L):
                    wb = wbp.tile([128, TILE], F32, tag="wb")
                    nc.tensor.matmul(wb[:], rowones_sb[:, l, :], w8[:],
                                     start=True, stop=True)
                    y = ys.tile([128, TILE], BF16, tag="y")
                    if l == lp:
                        # y = (wb + 1) * proj[lp]  (folds in the projlp term)
                        nc.vector.scalar_tensor_tensor(
                            y[:], wb[:], 1.0, proj[:, l, :], ALU.add, ALU.mult)
                    else:
                        nc.vector.tensor_mul(y[:], proj[:, l, :], wb[:])
                    nc.tensor.matmul(agg[:], ident_sb[:], y[:],
                                     start=(l == 0), stop=(l == L - 1))
                ot = outs.tile([128, TILE], BF16, tag="ot")
                nc.scalar.activation(ot[:], agg[:], AF.Copy, bias=0.0,
                                     scale=1.0)
                nc.sync.dma_start(out[:, ts(t, TILE)], ot[:])


def _build(lp: int, logis_b: float, nt: int = NT):
    nc = bacc.Bacc("TRN2", target_bir_lowering=False, debug=False,
                   num_devices=CORES)
    ins = {
        "xt": nc.dram_tensor("xt", [nt, 128, L, TILE], BF16,
                             kind="ExternalInput").ap(),
        "trans": nc.dram_tensor("trans", [128, 128], BF16,
                                kind="ExternalInput").ap(),
        "v8oh": nc.dram_tensor("v8oh", [128, L, 8], BF16,
                               kind="ExternalInput").ap(),
        "rowones": nc.dram_tensor("rowones", [8, L, 128], BF16,
                                  kind="ExternalInput").ap(),
        "sumsel": nc.dram_tensor("sumsel", [8, GRP, 65], BF16,
                                 kind="ExternalInput").ap(),
        "onesrow": nc.dram_tensor("onesrow", [128, 8], BF16,
                                  kind="ExternalInput").ap(),
        "ident": nc.dram_tensor("ident", [128, 128], BF16,
                                kind="ExternalInput").ap(),
        "biasc": nc.dram_tensor("biasc", [128, 1], F32,
                                kind="ExternalInput").ap(),
    }
    out = nc.dram_tensor("out", [128, nt * TILE], BF16,
                         kind="ExternalOutput").ap()
    with tile.TileContext(nc) as tc:
        _body(tc, out, ins, lp, logis_b, nt)
    nc.compile()
    return nc


def _host_prep(inputs):
    nf = np.asarray(inputs["node_features"], np.float32)      # [L, N, F]
    trans = np.asarray(inputs["trans"], np.float32)           # [F, F]
    biasv = np.asarray(inputs["bias"], np.float32).reshape(F) # [F]
    theta = np.asarray(inputs["theta"], np.float32)           # [L, F, F]
    lw = np.asarray(inputs["logis_w"], np.float32).reshape(1, F)
    lb = float(np.asarray(inputs["logis_b"], np.float32).reshape(-1)[0])
    lp = int(np.asarray(inputs["layer_predict"]).reshape(-1)[0])

    v8 = theta @ lw[0]                                        # [L, F]
    v8oh = np.zeros((128, L, 8), np.float32)
    for l in range(L):
        v8oh[:, l, l] = v8[l]
    rowones = np.zeros((8, L, 128), np.float32)
    for l in range(L):
        rowones[l, l, :] = 1.0
    sumsel = np.zeros((8, GRP, 65), np.float32)
    for k in range(GRP):
        sumsel[:, k, 32 * k] = 1.0

    consts = {
        "trans": trans.astype(BF),
        "v8oh": v8oh.astype(BF),
        "rowones": rowones.astype(BF),
        "sumsel": sumsel.astype(BF),
        "onesrow": np.ones((128, 8), np.float32).astype(BF),
        "ident": np.eye(128, dtype=np.float32).astype(BF),
        "biasc": np.ascontiguousarray(biasv.reshape(128, 1)),
    }

    # per-core packed node features: [NT, F, L, TILE] bf16
    nfb = nf.astype(BF)                                       # [L, N, F]

    def prep_core(c):
        sl = nfb[:, c * NS:(c + 1) * NS, :]                   # [L, NS, F]
        xt = np.transpose(sl, (2, 0, 1))                      # [F, L, NS]
        if NSP != NS:
            xt = np.concatenate(
                [xt, np.zeros((F, L, NSP - NS), BF)], axis=2)
        xt = np.ascontiguousarray(
            np.transpose(xt.reshape(F, L, NT, TILE), (2, 0, 1, 3)))
        return {"xt": xt, **consts}

    with ThreadPoolExecutor(CORES) as ex:
        in_maps = list(ex.map(prep_core, range(CORES)))
    return in_maps, lp, lb


_cache = {}


def _run(inputs, trace=False):
    from concourse.bass_utils import run_bass_kernel_spmd

    in_maps, lp, lb = _host_prep(inputs)
    key = (lp, round(lb, 8))
    if key not in _cache:
        _cache[key] = _build(lp, lb)
    nc = _cache[key]

    res = run_bass_kernel_spmd(nc, in_maps, core_ids=list(range(CORES)),
                               trace=trace)
    full = np.empty((N, F), np.float32)

    def fetch(c):
        o = np.asarray(res.results[c]["out"], dtype=np.float32)  # [128, NSP]
        full[c * NS:(c + 1) * NS] = o[:, :NS].T

    with ThreadPoolExecutor(CORES) as ex:
        list(ex.map(fetch, range(CORES)))
    return full, res


def kernel(**inputs) -> np.ndarray:
    out, _ = _run(inputs, trace=False)
    return out
